# revision 34
# baseline (speedup 1.0000x reference)
"""Trainium2 Bass kernel for nn_MultiHeadAttention_60078002536549.

Dense transformer block:
    att  = softmax(Q K^T / sqrt(64)) V          (B=2, H=16, N=2048, HD=64)
    x1   = x + att_concat                        (B, N, D=1024)
    out  = x1 + gelu(LN(x1) @ w1 + b1) @ w2 + b2 (FF=4096)

Sharding: tokens are sharded across the 8 cores (core i handles batch
i//4, token rows [512*(i%4), 512*(i%4+1))).  Each core loads the full
K/V of its batch and the full FFN weights; no collectives.

v2 layout strategy: all activations live feature-major ([feature
partitions, token free-dim]) end-to-end.  The host pre-transposes x, Q
and K (and transposes y back), so the kernel needs NO on-chip PE
transposes.  Weights are cast fp32->bf16 during DMA (SWDGE cast), w1 is
fully SBUF-resident and prefetched during attention.  Scores run as
bf16 row-tiled concurrent head pairs; AV runs as fp8e4 DoubleRow
matmuls (2 k-chunks per instruction) with a ones-column appended to V
for the softmax denominators.  Softmax exp is batched 2 chunks per ACT
call; LN and normalization run on DVE.
"""

import sys

for _p in ("/opt/trn_rl_repo",):
    if _p not in sys.path:
        sys.path.insert(0, _p)

import numpy as np

import concourse.bass as bass
import concourse.mybir as mybir
import concourse.tile as tile
from concourse.bass import ts
from concourse.bass_utils import run_bass_kernel_spmd

F32 = mybir.dt.float32
F32R = mybir.dt.float32r
BF16 = mybir.dt.bfloat16
F8 = mybir.dt.float8e4
AF = mybir.ActivationFunctionType
ALU = mybir.AluOpType
DR = mybir.MatmulPerfMode.DoubleRow

B, H, N, HD, D, FF = 2, 16, 2048, 64, 1024, 4096
NCORES = 8
TOK = (B * N) // NCORES          # 512 tokens per core
SCALE = float(1.0 / np.sqrt(HD))
EPS = 1e-5

KC = N // 128                    # 16 k-token chunks
NCC = KC // 2                    # 8 double-chunks
DC = D // 128                    # 8 feature chunks
FC = FF // 128                   # 32 hidden chunks
NPAIR = H // 2                   # 8 head pairs
DEBUG_X1 = False


def r32(ap):
    return ap.bitcast(F32R)


def build_program(split_waits=True):
    nc = bass.Bass()

    # Host-pretransposed inputs (feature-major).
    xt = nc.declare_dram_parameter("xt", [D, TOK], F32, isOutput=False)
    qt = nc.declare_dram_parameter("qt", [NPAIR, 128, TOK], F32, isOutput=False)
    kt = nc.declare_dram_parameter("kt", [NPAIR, 128, N], F32, isOutput=False)
    vs = nc.declare_dram_parameter("vs", [H, 128, KC, HD], F32,
                                   isOutput=False)
    w1 = nc.declare_dram_parameter("w1", [D, FF], F32, isOutput=False)
    b1 = nc.declare_dram_parameter("b1", [FF], F32, isOutput=False)
    w2 = nc.declare_dram_parameter("w2", [FF, D], F32, isOutput=False)
    b2 = nc.declare_dram_parameter("b2", [D], F32, isOutput=False)
    lnw = nc.declare_dram_parameter("lnw", [D], F32, isOutput=False)
    lnb = nc.declare_dram_parameter("lnb", [D], F32, isOutput=False)
    y = nc.declare_dram_parameter("y", [D, TOK], F32, isOutput=True)
    x1dbg = (nc.declare_dram_parameter("x1dbg", [DC, 128, TOK], F32,
                                       isOutput=True) if DEBUG_X1 else None)

    xt_v = xt[:].rearrange("(dc p) t -> p dc t", p=128)        # [128, 8, TOK]
    y_v = y[:].rearrange("(dd p) t -> p dd t", p=128)          # [128, 8, TOK]
    w1_v = w1[:].rearrange("(dc p) f -> p dc f", p=128)        # [128, 8, FF]
    w2_v = w2[:].rearrange("(fc p) d -> p fc d", p=128)        # [128, 32, D]

    with tile.TileContext(nc) as tc:
        build_tile_kernel(nc, tc, xt_v, qt, kt, vs, w1_v, b1, w2_v, b2,
                          lnw, lnb, y_v, x1dbg)
    if split_waits:
        _split_matmul_waits(nc)
    return nc


def _split_matmul_waits(nc):
    """This walrus build accepts only one sync wait per compute engine
    instruction; move extra waits onto a NoOp inserted right before it on
    the same engine."""
    for f in nc.m.functions:
        for blk in f.blocks:
            new = []
            for inst in blk.instructions:
                si = inst.sync_info
                if si is not None and len(si.on_wait) > 1:
                    waits = list(si.on_wait)
                    for w in waits[:-1]:
                        new.append(mybir.InstNoOp(
                            name=f"waitsplit_{nc.next_id()}",
                            engine=inst.engine, ins=[], outs=[],
                            sync_info=mybir.SyncInfo(on_wait=[w],
                                                     on_update=[])))
                    inst.sync_info = mybir.SyncInfo(
                        on_wait=waits[-1:], on_update=list(si.on_update))
                new.append(inst)
            blk.instructions[:] = new


def build_tile_kernel(nc, tc, xt_v, qt, kt, vs, w1_v, b1, w2_v, b2,
                      lnw, lnb, y_v, x1dbg=None):
    from contextlib import ExitStack

    est = ExitStack()
    singles = est.enter_context(tc.tile_pool(name="singles", bufs=1))
    persist = est.enter_context(tc.tile_pool(name="persist", bufs=1))
    dram_p = est.enter_context(tc.tile_pool(name="drp", bufs=2, space="DRAM"))

    # ---- constants / small params ----
    ones_f32 = singles.tile([128, 1], F32, tag="ones_f32")
    nc.vector.memset(ones_f32, 1.0)
    ones_col = singles.tile([128, 1], F32R, tag="ones_col")
    nc.vector.tensor_copy(out=ones_col, in_=ones_f32)
    eps_t = singles.tile([1, 1], F32, tag="eps")
    nc.vector.memset(eps_t, EPS)
    eshift = singles.tile([128, 1], F32, tag="eshift")
    nc.vector.memset(eshift, -4.25)
    lnw_sb = singles.tile([128, DC], F32, tag="lnw")
    lnb_sb = singles.tile([128, DC], F32, tag="lnb")
    b2_sb = singles.tile([128, DC], F32, tag="b2")
    b1_sb = singles.tile([128, FC], F32, tag="b1")

    # ---- w2 fully resident in bf16 (cast during DMA; the per-chunk
    # loads are emitted inside the pair loop so the SWDGE queue serves
    # pair 0's K/Q/V first) ----
    w2b = persist.tile([128, FC, D], BF16, tag="w2b")

    # x staged feature-major (DMA emitted after pair-0/1 staging; x is
    # first needed only at pair 0's residual add)
    xT = persist.tile([128, DC, TOK], F32, tag="xT")

    # x1 = x + attention output (residual stream), f32r for the stats mms
    x1T = [persist.tile([128, TOK], F32R, name=f"x1T{j}", tag=f"x1T{j}")
           for j in range(DC)]

    # =================== attention ===================
    att_est = ExitStack()
    kst_p = att_est.enter_context(tc.tile_pool(name="kst", bufs=2))
    qst_p = att_est.enter_context(tc.tile_pool(name="qst", bufs=2))
    vst_p = att_est.enter_context(tc.tile_pool(name="vst", bufs=2))
    exp_p = att_est.enter_context(tc.tile_pool(name="expp", bufs=3))
    nrm_p = att_est.enter_context(tc.tile_pool(name="nrm", bufs=2))
    sq_p = att_est.enter_context(tc.tile_pool(name="sqp", bufs=2))
    wst_p = att_est.enter_context(tc.tile_pool(name="wst", bufs=2))
    s_ps = att_est.enter_context(tc.tile_pool(name="s_ps", bufs=2, space="PSUM"))
    att_ps = att_est.enter_context(tc.tile_pool(name="att_ps", bufs=1, space="PSUM"))

    st_est = ExitStack()
    st_ps = st_est.enter_context(tc.tile_pool(name="st_ps", bufs=1, space="PSUM"))
    stats = st_ps.tile([1, 2, TOK], F32, tag="stats")

    def emit_stats(jprev, sq_tile):
        nc.tensor.matmul(stats[:, 0, :], ones_col, r32(x1T[jprev]),
                         start=(jprev == 0), stop=(jprev == NPAIR - 1))
        nc.tensor.matmul(stats[:, 1, :], ones_col, r32(sq_tile),
                         start=(jprev == 0), stop=(jprev == NPAIR - 1))

    # K/Q/V staged f32 on the HWDGE queues, then DVE-cast (bf16 for the
    # score operands, fp8e4 for the DoubleRow AV).  Staging and casts for
    # pair j+1 are emitted during pair j so the DVE work overlaps the
    # exp stream of the previous pair.
    # K/Q/V load through the SWDGE cast queue (bf16 / fp8 directly);
    # with the weights kept off that queue it has ample bandwidth.
    def stage_pair(j):
        ha, hb = 2 * j, 2 * j + 1
        kt_sb = kst_p.tile([128, N], BF16, tag="ktb")
        nc.gpsimd.dma_start(out=kt_sb, in_=kt[j])
        qt_sb = qst_p.tile([128, TOK], BF16, tag="qtb")
        nc.gpsimd.dma_start(out=qt_sb, in_=qt[j])
        # V fp8 with a ones column (softmax denominator); free dim padded
        # to 80 so the DoubleRow ko-stride is 16-aligned
        v8a = vst_p.tile([128, KC, 80], F8, tag="v8a")
        v8b = vst_p.tile([128, KC, 80], F8, tag="v8b")
        nc.gpsimd.dma_start(out=v8a[:, :, 0:HD], in_=vs[ha])
        nc.gpsimd.dma_start(out=v8b[:, :, 0:HD], in_=vs[hb])
        nc.vector.memset(v8a[:, :, HD:HD + 1], 1.0)
        nc.vector.memset(v8b[:, :, HD:HD + 1], 1.0)
        return kt_sb, qt_sb, v8a, v8b

    ready = stage_pair(0)
    # PE warm-up: ~3.4us of back-to-back dummy matmuls into the stats
    # bank flips the HAM clock-gate to full rate before the attention
    # stream starts; the real stats accumulation later begins with
    # start=True, so the garbage is overwritten.
    ones_bf = singles.tile([128, 1], BF16, tag="ones_bf")
    nc.vector.memset(ones_bf, 1.0)
    for w in range(16):
        nc.tensor.matmul(stats[:, 0, :], ones_bf, ready[0][:, 0:TOK])
    # small params, deferred so they sit behind pair-0 K/Q/V in no queue
    nc.sync.dma_start(out=lnw_sb, in_=lnw[:].rearrange("(c p) -> p c", p=128))
    nc.sync.dma_start(out=lnb_sb, in_=lnb[:].rearrange("(c p) -> p c", p=128))
    nc.scalar.dma_start(out=b2_sb, in_=b2[:].rearrange("(c p) -> p c", p=128))
    nc.scalar.dma_start(out=b1_sb, in_=b1[:].rearrange("(c p) -> p c", p=128))

    pend_sq = None
    for j in range(NPAIR):
        if j + 1 < NPAIR:
            ready_next = stage_pair(j + 1)
        kt_sb, qt_sb, v8a, v8b = ready
        nc.scalar.dma_start(out=xT[:, j, :], in_=xt_v[:, j, :])

        # prefetch one w2 column-chunk per pair through the HWDGE queues
        # (f32 staged, DVE-cast to the resident bf16 buffer)
        w2s = wst_p.tile([128, FC, 128], F32, tag="w2s")
        nc.sync.dma_start(out=w2s[:, 0:FC // 2, :],
                          in_=w2_v[:, 0:FC // 2, ts(j, 128)])
        nc.scalar.dma_start(out=w2s[:, FC // 2:, :],
                            in_=w2_v[:, FC // 2:, ts(j, 128)])
        nc.vector.tensor_copy(out=w2b[:, :, ts(j, 128)], in_=w2s)

        if pend_sq is not None:
            sq_prev = sq_p.tile([128, TOK], F32R, tag="sq")
            nc.vector.tensor_mul(sq_prev, x1T[j - 1], x1T[j - 1])
            pend_sq = sq_prev

        # ---- scores + exp + AV (fp8 DoubleRow), per-chunk pipeline ----
        att_a = att_ps.tile([HD + 1, TOK], F32, tag="att_a")
        att_b = att_ps.tile([HD + 1, TOK], F32, tag="att_b")
        # exp(scores) in fp8, one tile per double-chunk laid out
        # [parity, head, tok]; the AV DoubleRow matmuls contract over
        # both parities of a double-chunk at once
        # exp shifted down so fp8e4 (max 240) cannot overflow at ~6-sigma
        # scores; the constant factor cancels between numerator and the
        # ones-column denominator.
        def exp_c(c, s, e2):
            nc.scalar.activation(e2[:, c & 1, :, :], s, AF.Exp, scale=SCALE,
                                 bias=eshift)

        def av_g(g, e2):
            nc.tensor.matmul(att_a, v8a[:, 2 * g:2 * g + 2, 0:HD + 1],
                             e2[:, :, 0, :], start=(g == 0),
                             stop=(g == NCC - 1), perf_mode=DR)
            nc.tensor.matmul(att_b, v8b[:, 2 * g:2 * g + 2, 0:HD + 1],
                             e2[:, :, 1, :], start=(g == 0),
                             stop=(g == NCC - 1), perf_mode=DR)

        pend = None
        e2 = None
        for c in range(KC):
            if c % 2 == 0:
                e2 = exp_p.tile([128, 2, 2, TOK], F8, tag="e2")
            s = s_ps.tile([128, 2, TOK], F32, tag="s")
            nc.tensor.matmul(s[:, 0, :], kt_sb[0:64, ts(c, 128)],
                             qt_sb[0:64, :], tile_position=(0, 0))
            nc.tensor.matmul(s[:, 1, :], kt_sb[64:128, ts(c, 128)],
                             qt_sb[64:128, :], tile_position=(64, 0))
            if pend is not None:
                exp_c(*pend)
                if pend[0] & 1:
                    av_g(pend[0] // 2, pend[2])
            pend = (c, s, e2)
        exp_c(*pend)
        av_g(NCC - 1, e2)

        # ---- evacuate att PSUM to SBUF immediately (frees the att
        # banks so the next pair's AV accumulation can start), then
        # normalize by the ones-row denominators with a DVE divide ----
        asb = nrm_p.tile([HD + 1, TOK], F32, tag="asb")
        bsb = nrm_p.tile([HD + 1, TOK], F32, tag="bsb")
        nc.vector.tensor_copy(out=asb, in_=att_a)
        nc.vector.tensor_copy(out=bsb, in_=att_b)
        # denominators bounce through DRAM reshaped to [128, 8] so the
        # reciprocal uses all 128 DVE lanes (a [1, 512] reciprocal is
        # single-lane and costs ~4us)
        bcd = dram_p.tile([2, TOK], F32, tag="bcd")
        nc.sync.dma_start(out=bcd[0:1, :], in_=asb[HD:HD + 1, :])
        nc.sync.dma_start(out=bcd[1:2, :], in_=bsb[HD:HD + 1, :])
        rcp = nrm_p.tile([128, 2 * TOK // 128], F32, tag="rcp")
        nc.sync.dma_start(
            out=rcp, in_=bcd[:, :].rearrange("a (g c) -> (a g) c", c=2 * TOK // 128))
        nc.vector.reciprocal(rcp, rcp)
        bcd2 = dram_p.tile([2, TOK], F32, tag="bcd2")
        nc.sync.dma_start(
            out=bcd2[:, :].rearrange("a (g c) -> (a g) c", c=2 * TOK // 128),
            in_=rcp)
        bca = nrm_p.tile([64, TOK], F32, tag="bca")
        bcb = nrm_p.tile([64, TOK], F32, tag="bcb")
        nc.sync.dma_start(out=bca, in_=bcd2[0:1, :].to_broadcast((64, TOK)))
        nc.sync.dma_start(out=bcb, in_=bcd2[1:2, :].to_broadcast((64, TOK)))
        natt = nrm_p.tile([128, TOK], F32, tag="natt")
        tmpb = nrm_p.tile([64, TOK], F32, tag="tmpb")
        nc.vector.tensor_mul(natt[0:64, :], asb[0:HD, :], bca)
        nc.vector.tensor_mul(tmpb, bsb[0:HD, :], bcb)
        nc.sync.dma_start(out=natt[64:128, :], in_=tmpb)
        nc.vector.tensor_add(x1T[j], natt, xT[:, j, :])

        # flush the previous pair's layer-norm stats (deferred so the PE
        # queue at the pair boundary is not blocked behind the DVE square)
        if pend_sq is not None:
            emit_stats(j - 1, pend_sq)
        pend_sq = True
        if j + 1 < NPAIR:
            ready = ready_next

    sq_last = sq_p.tile([128, TOK], F32R, tag="sq")
    nc.vector.tensor_mul(sq_last, x1T[NPAIR - 1], x1T[NPAIR - 1])
    emit_stats(NPAIR - 1, sq_last)

    if DEBUG_X1:
        for j in range(DC):
            nc.sync.dma_start(out=x1dbg[j].bitcast(F32R), in_=x1T[j])

    # ---- layer-norm scalars ----
    mu = persist.tile([1, TOK], F32, tag="mu")
    msq = persist.tile([1, TOK], F32, tag="msq")
    var = persist.tile([1, TOK], F32, tag="var")
    rstd = persist.tile([1, TOK], F32, tag="rstd")
    nc.vector.tensor_scalar_mul(mu, stats[:, 0, :], 1.0 / D)
    nc.vector.tensor_scalar_mul(msq, stats[:, 1, :], 1.0 / D)
    st_est.close()
    nc.vector.tensor_mul(var, mu, mu)
    nc.vector.tensor_sub(var, msq, var)
    # rstd = exp(-0.5 * ln(var + eps)) -- stays within the ln/exp table set
    nc.scalar.activation(var, var, AF.Ln, bias=eps_t)
    nc.scalar.activation(rstd, var, AF.Exp, scale=-0.5)

    mu_b = persist.tile([128, TOK], F32, tag="mu_b")
    rstd_b = persist.tile([128, TOK], F32, tag="rstd_b")
    lnd = dram_p.tile([2, TOK], F32, tag="lnd")
    nc.sync.dma_start(out=lnd[0:1, :], in_=mu)
    nc.sync.dma_start(out=lnd[1:2, :], in_=rstd)
    nc.sync.dma_start(out=mu_b, in_=lnd[0:1, :].to_broadcast((128, TOK)))
    nc.sync.dma_start(out=rstd_b, in_=lnd[1:2, :].to_broadcast((128, TOK)))

    att_est.close()

    # =================== FFN ===================
    ffn_est = ExitStack()
    mm_ps = ffn_est.enter_context(tc.tile_pool(name="mm_ps", bufs=4, space="PSUM"))
    ln_p = ffn_est.enter_context(tc.tile_pool(name="ln", bufs=2))
    g_p = ffn_est.enter_context(tc.tile_pool(name="gp", bufs=1))
    h_p = ffn_est.enter_context(tc.tile_pool(name="hp", bufs=1))
    o_p = ffn_est.enter_context(tc.tile_pool(name="op", bufs=2))

    w1_p = ffn_est.enter_context(tc.tile_pool(name="w1p", bufs=4))

    # LN on DVE: h = ((x1 - mu) * lnw) * rstd + lnb, bf16
    hT = [h_p.tile([128, TOK], BF16, name=f"hT{j}", tag=f"hT{j}")
          for j in range(DC)]
    for j in range(DC):
        t = ln_p.tile([128, TOK], F32, tag="lnt")
        nc.vector.tensor_sub(t, x1T[j], mu_b)
        nc.vector.scalar_tensor_tensor(
            t, t, lnw_sb[:, j:j + 1], rstd_b, op0=ALU.mult, op1=ALU.mult)
        nc.vector.tensor_scalar_add(hT[j], t, lnb_sb[:, j:j + 1])

    gT = [g_p.tile([128, TOK], BF16, name=f"gT{f}", tag=f"gT{f}")
          for f in range(FC)]
    for f in range(FC):
        w1s = w1_p.tile([128, DC, 128], F32, tag="w1s")
        nc.sync.dma_start(out=w1s[:, 0:DC // 2, :],
                          in_=w1_v[:, 0:DC // 2, ts(f, 128)])
        nc.scalar.dma_start(out=w1s[:, DC // 2:, :],
                            in_=w1_v[:, DC // 2:, ts(f, 128)])
        w1c = w1_p.tile([128, DC, 128], BF16, tag="w1c")
        nc.vector.tensor_copy(out=w1c, in_=w1s)
        ps = mm_ps.tile([128, TOK], F32, tag="mm")
        for dc in range(DC):
            nc.tensor.matmul(ps, w1c[:, dc, :], hT[dc],
                             start=(dc == 0), stop=(dc == DC - 1))
        nc.scalar.activation(gT[f], ps, AF.Gelu, bias=b1_sb[:, f:f + 1])

    for dd in range(DC):
        ps = mm_ps.tile([128, TOK], F32, tag="mm")
        for fc in range(FC):
            nc.tensor.matmul(ps, w2b[:, fc, ts(dd, 128)], gT[fc],
                             start=(fc == 0), stop=(fc == FC - 1))
        yt = o_p.tile([128, TOK], F32, tag="yt")
        nc.vector.scalar_tensor_tensor(
            yt, ps, b2_sb[:, dd:dd + 1], x1T[dd], op0=ALU.add, op1=ALU.add)
        nc.sync.dma_start(out=y_v[:, dd, :], in_=yt)

    ffn_est.close()
    est.close()


_PROGRAMS = {}


def get_program(split_waits=True):
    if split_waits not in _PROGRAMS:
        _PROGRAMS[split_waits] = build_program(split_waits)
    return _PROGRAMS[split_waits]


def make_in_maps(x, image_q, image_k, image_v, ln_w, ln_b, w1, b1, w2, b2):
    asf = lambda a: np.ascontiguousarray(np.asarray(a, dtype=np.float32))
    x = np.asarray(x, dtype=np.float32)
    image_q = np.asarray(image_q, dtype=np.float32)
    image_k = np.asarray(image_k, dtype=np.float32)
    image_v = np.asarray(image_v, dtype=np.float32)
    shared = {
        "w1": asf(w1), "b1": asf(b1), "w2": asf(w2), "b2": asf(b2),
        "lnw": asf(ln_w), "lnb": asf(ln_b),
    }
    # per batch: feature-major K pairs [NPAIR, 128, N]
    ktb = [asf(image_k[b].transpose(0, 2, 1).reshape(NPAIR, 128, N))
           for b in range(B)]
    # V chunk layout matching contiguous score chunks: [H, p, c, d],
    # token = c*128 + p
    vb = [asf(image_v[b].reshape(H, KC, 128, HD).transpose(0, 2, 1, 3))
          for b in range(B)]
    in_maps = []
    for core in range(NCORES):
        b, r = divmod(core, NCORES // B)
        rows = slice(TOK * r, TOK * (r + 1))
        in_maps.append({
            "xt": asf(x[b, rows].T),
            "qt": asf(image_q[b, :, rows].transpose(0, 2, 1).reshape(
                NPAIR, 128, TOK)),
            "kt": ktb[b],
            "vs": vb[b],
            **shared,
        })
    return in_maps


def run_cores(in_maps, trace=False, **kw):
    nc = get_program()
    return run_bass_kernel_spmd(nc, in_maps, core_ids=list(range(NCORES)),
                                trace=trace, **kw)


def kernel(x, image_q, image_k, image_v, ln_w, ln_b, w1, b1, w2, b2):
    in_maps = make_in_maps(x, image_q, image_k, image_v, ln_w, ln_b,
                           w1, b1, w2, b2)
    res = run_cores(in_maps)
    out = np.empty((B, N, D), dtype=np.float32)
    for core in range(NCORES):
        b, r = divmod(core, NCORES // B)
        out[b, TOK * r:TOK * (r + 1)] = res.results[core]["y"].T
    return out


# revision 35
# speedup vs baseline: 1.0219x; 1.0219x over previous
"""Trainium2 Bass kernel for nn_MultiHeadAttention_60078002536549.

Dense transformer block:
    att  = softmax(Q K^T / sqrt(64)) V          (B=2, H=16, N=2048, HD=64)
    x1   = x + att_concat                        (B, N, D=1024)
    out  = x1 + gelu(LN(x1) @ w1 + b1) @ w2 + b2 (FF=4096)

Sharding: tokens are sharded across the 8 cores (core i handles batch
i//4, token rows [512*(i%4), 512*(i%4+1))).  Each core loads the full
K/V of its batch and the full FFN weights; no collectives.

v2 layout strategy: all activations live feature-major ([feature
partitions, token free-dim]) end-to-end.  The host pre-transposes x, Q
and K (and transposes y back), so the kernel needs NO on-chip PE
transposes.  Weights are cast fp32->bf16 during DMA (SWDGE cast), w1 is
fully SBUF-resident and prefetched during attention.  Scores run as
bf16 row-tiled concurrent head pairs; AV runs as fp8e4 DoubleRow
matmuls (2 k-chunks per instruction) with a ones-column appended to V
for the softmax denominators.  Softmax exp is batched 2 chunks per ACT
call; LN and normalization run on DVE.
"""

import sys

for _p in ("/opt/trn_rl_repo",):
    if _p not in sys.path:
        sys.path.insert(0, _p)

import numpy as np

import concourse.bass as bass
import concourse.mybir as mybir
import concourse.tile as tile
from concourse.bass import ts
from concourse.bass_utils import run_bass_kernel_spmd

F32 = mybir.dt.float32
F32R = mybir.dt.float32r
BF16 = mybir.dt.bfloat16
F8 = mybir.dt.float8e4
AF = mybir.ActivationFunctionType
ALU = mybir.AluOpType
DR = mybir.MatmulPerfMode.DoubleRow

B, H, N, HD, D, FF = 2, 16, 2048, 64, 1024, 4096
NCORES = 8
TOK = (B * N) // NCORES          # 512 tokens per core
SCALE = float(1.0 / np.sqrt(HD))
EPS = 1e-5

KC = N // 128                    # 16 k-token chunks
NCC = KC // 2                    # 8 double-chunks
DC = D // 128                    # 8 feature chunks
FC = FF // 128                   # 32 hidden chunks
NPAIR = H // 2                   # 8 head pairs
DEBUG_X1 = False


def r32(ap):
    return ap.bitcast(F32R)


def build_program(split_waits=True):
    nc = bass.Bass()

    # Host-pretransposed inputs (feature-major).
    xt = nc.declare_dram_parameter("xt", [D, TOK], F32, isOutput=False)
    qt = nc.declare_dram_parameter("qt", [NPAIR, 128, TOK], F32, isOutput=False)
    kt = nc.declare_dram_parameter("kt", [NPAIR, 128, N], F32, isOutput=False)
    vs = nc.declare_dram_parameter("vs", [H, 128, KC, HD], F32,
                                   isOutput=False)
    w1 = nc.declare_dram_parameter("w1", [D, FF], F32, isOutput=False)
    b1 = nc.declare_dram_parameter("b1", [FF], F32, isOutput=False)
    w2 = nc.declare_dram_parameter("w2", [FF, D], F32, isOutput=False)
    b2 = nc.declare_dram_parameter("b2", [D], F32, isOutput=False)
    lnw = nc.declare_dram_parameter("lnw", [D], F32, isOutput=False)
    lnb = nc.declare_dram_parameter("lnb", [D], F32, isOutput=False)
    y = nc.declare_dram_parameter("y", [D, TOK], F32, isOutput=True)
    x1dbg = (nc.declare_dram_parameter("x1dbg", [DC, 128, TOK], F32,
                                       isOutput=True) if DEBUG_X1 else None)

    xt_v = xt[:].rearrange("(dc p) t -> p dc t", p=128)        # [128, 8, TOK]
    y_v = y[:].rearrange("(dd p) t -> p dd t", p=128)          # [128, 8, TOK]
    w1_v = w1[:].rearrange("(dc p) f -> p dc f", p=128)        # [128, 8, FF]
    w2_v = w2[:].rearrange("(fc p) d -> p fc d", p=128)        # [128, 32, D]

    with tile.TileContext(nc) as tc:
        build_tile_kernel(nc, tc, xt_v, qt, kt, vs, w1_v, b1, w2_v, b2,
                          lnw, lnb, y_v, x1dbg)
    if split_waits:
        _split_matmul_waits(nc)
    return nc


def _split_matmul_waits(nc):
    """This walrus build accepts only one sync wait per compute engine
    instruction; move extra waits onto a NoOp inserted right before it on
    the same engine."""
    for f in nc.m.functions:
        for blk in f.blocks:
            new = []
            for inst in blk.instructions:
                si = inst.sync_info
                if si is not None and len(si.on_wait) > 1:
                    waits = list(si.on_wait)
                    for w in waits[:-1]:
                        new.append(mybir.InstNoOp(
                            name=f"waitsplit_{nc.next_id()}",
                            engine=inst.engine, ins=[], outs=[],
                            sync_info=mybir.SyncInfo(on_wait=[w],
                                                     on_update=[])))
                    inst.sync_info = mybir.SyncInfo(
                        on_wait=waits[-1:], on_update=list(si.on_update))
                new.append(inst)
            blk.instructions[:] = new


def build_tile_kernel(nc, tc, xt_v, qt, kt, vs, w1_v, b1, w2_v, b2,
                      lnw, lnb, y_v, x1dbg=None):
    from contextlib import ExitStack

    est = ExitStack()
    singles = est.enter_context(tc.tile_pool(name="singles", bufs=1))
    persist = est.enter_context(tc.tile_pool(name="persist", bufs=1))
    dram_p = est.enter_context(tc.tile_pool(name="drp", bufs=2, space="DRAM"))

    # ---- constants / small params ----
    ones_f32 = singles.tile([128, 1], F32, tag="ones_f32")
    nc.vector.memset(ones_f32, 1.0)
    ones_col = singles.tile([128, 1], F32R, tag="ones_col")
    nc.vector.tensor_copy(out=ones_col, in_=ones_f32)
    eps_t = singles.tile([1, 1], F32, tag="eps")
    nc.vector.memset(eps_t, EPS)
    eshift = singles.tile([128, 1], F32, tag="eshift")
    nc.vector.memset(eshift, -4.25)
    lnw_sb = singles.tile([128, DC], F32, tag="lnw")
    lnb_sb = singles.tile([128, DC], F32, tag="lnb")
    b2_sb = singles.tile([128, DC], F32, tag="b2")
    b1_sb = singles.tile([128, FC], F32, tag="b1")

    # ---- w2 fully resident in bf16 (cast during DMA; the per-chunk
    # loads are emitted inside the pair loop so the SWDGE queue serves
    # pair 0's K/Q/V first) ----
    w2b = persist.tile([128, FC, D], BF16, tag="w2b")

    # x staged feature-major (DMA emitted after pair-0/1 staging; x is
    # first needed only at pair 0's residual add)
    xT = persist.tile([128, DC, TOK], F32, tag="xT")

    # x1 = x + attention output (residual stream), f32r for the stats mms
    x1T = [persist.tile([128, TOK], F32R, name=f"x1T{j}", tag=f"x1T{j}")
           for j in range(DC)]

    # =================== attention ===================
    att_est = ExitStack()
    kst_p = att_est.enter_context(tc.tile_pool(name="kst", bufs=2))
    qst_p = att_est.enter_context(tc.tile_pool(name="qst", bufs=2))
    vst_p = att_est.enter_context(tc.tile_pool(name="vst", bufs=2))
    exp_p = att_est.enter_context(tc.tile_pool(name="expp", bufs=3))
    nrm_p = att_est.enter_context(tc.tile_pool(name="nrm", bufs=2))
    sq_p = att_est.enter_context(tc.tile_pool(name="sqp", bufs=2))
    wst_p = att_est.enter_context(tc.tile_pool(name="wst", bufs=2))
    s_ps = att_est.enter_context(tc.tile_pool(name="s_ps", bufs=2, space="PSUM"))
    att_ps = att_est.enter_context(tc.tile_pool(name="att_ps", bufs=1, space="PSUM"))

    st_est = ExitStack()
    st_ps = st_est.enter_context(tc.tile_pool(name="st_ps", bufs=1, space="PSUM"))
    stats = st_ps.tile([1, 2, TOK], F32, tag="stats")

    def emit_stats(jprev, sq_tile):
        nc.tensor.matmul(stats[:, 0, :], ones_col, r32(x1T[jprev]),
                         start=(jprev == 0), stop=(jprev == NPAIR - 1))
        nc.tensor.matmul(stats[:, 1, :], ones_col, r32(sq_tile),
                         start=(jprev == 0), stop=(jprev == NPAIR - 1))

    # K/Q/V staged f32 on the HWDGE queues, then DVE-cast (bf16 for the
    # score operands, fp8e4 for the DoubleRow AV).  Staging and casts for
    # pair j+1 are emitted during pair j so the DVE work overlaps the
    # exp stream of the previous pair.
    # K/Q/V load through the SWDGE cast queue (bf16 / fp8 directly);
    # with the weights kept off that queue it has ample bandwidth.
    def stage_pair(j):
        ha, hb = 2 * j, 2 * j + 1
        kt_sb = kst_p.tile([128, N], BF16, tag="ktb")
        nc.gpsimd.dma_start(out=kt_sb, in_=kt[j])
        qt_sb = qst_p.tile([128, TOK], BF16, tag="qtb")
        nc.gpsimd.dma_start(out=qt_sb, in_=qt[j])
        # V fp8 with a ones column (softmax denominator); free dim padded
        # to 80 so the DoubleRow ko-stride is 16-aligned
        v8a = vst_p.tile([128, KC, 80], F8, tag="v8a")
        v8b = vst_p.tile([128, KC, 80], F8, tag="v8b")
        nc.gpsimd.dma_start(out=v8a[:, :, 0:HD], in_=vs[ha])
        nc.gpsimd.dma_start(out=v8b[:, :, 0:HD], in_=vs[hb])
        nc.vector.memset(v8a[:, :, HD:HD + 1], 1.0)
        nc.vector.memset(v8b[:, :, HD:HD + 1], 1.0)
        return kt_sb, qt_sb, v8a, v8b

    ready = stage_pair(0)
    # PE warm-up: ~3.4us of back-to-back dummy matmuls into the stats
    # bank flips the HAM clock-gate to full rate before the attention
    # stream starts; the real stats accumulation later begins with
    # start=True, so the garbage is overwritten.
    ones_bf = singles.tile([128, 1], BF16, tag="ones_bf")
    nc.vector.memset(ones_bf, 1.0)
    wrm = singles.tile([128, TOK], BF16, tag="wrm")
    nc.vector.memset(wrm, 0.0)
    for w in range(16):
        nc.tensor.matmul(stats[:, 0, :], ones_bf, wrm)
    # small params, deferred so they sit behind pair-0 K/Q/V in no queue
    nc.sync.dma_start(out=lnw_sb, in_=lnw[:].rearrange("(c p) -> p c", p=128))
    nc.sync.dma_start(out=lnb_sb, in_=lnb[:].rearrange("(c p) -> p c", p=128))
    nc.scalar.dma_start(out=b2_sb, in_=b2[:].rearrange("(c p) -> p c", p=128))
    nc.scalar.dma_start(out=b1_sb, in_=b1[:].rearrange("(c p) -> p c", p=128))

    pend_sq = None
    for j in range(NPAIR):
        if j + 1 < NPAIR:
            ready_next = stage_pair(j + 1)
        kt_sb, qt_sb, v8a, v8b = ready
        nc.scalar.dma_start(out=xT[:, j, :], in_=xt_v[:, j, :])

        # prefetch one w2 column-chunk per pair through the HWDGE queues
        # (f32 staged, DVE-cast to the resident bf16 buffer)
        w2s = wst_p.tile([128, FC, 128], F32, tag="w2s")
        nc.sync.dma_start(out=w2s[:, 0:FC // 2, :],
                          in_=w2_v[:, 0:FC // 2, ts(j, 128)])
        nc.scalar.dma_start(out=w2s[:, FC // 2:, :],
                            in_=w2_v[:, FC // 2:, ts(j, 128)])
        nc.vector.tensor_copy(out=w2b[:, :, ts(j, 128)], in_=w2s)

        if pend_sq is not None:
            sq_prev = sq_p.tile([128, TOK], F32R, tag="sq")
            nc.vector.tensor_mul(sq_prev, x1T[j - 1], x1T[j - 1])
            pend_sq = sq_prev

        # ---- scores + exp + AV (fp8 DoubleRow), per-chunk pipeline ----
        att_a = att_ps.tile([HD + 1, TOK], F32, tag="att_a")
        att_b = att_ps.tile([HD + 1, TOK], F32, tag="att_b")
        # exp(scores) in fp8, one tile per double-chunk laid out
        # [parity, head, tok]; the AV DoubleRow matmuls contract over
        # both parities of a double-chunk at once
        # exp shifted down so fp8e4 (max 240) cannot overflow at ~6-sigma
        # scores; the constant factor cancels between numerator and the
        # ones-column denominator.
        def exp_c(c, s, e2):
            nc.scalar.activation(e2[:, c & 1, :, :], s, AF.Exp, scale=SCALE,
                                 bias=eshift)

        def av_g(g, e2):
            nc.tensor.matmul(att_a, v8a[:, 2 * g:2 * g + 2, 0:HD + 1],
                             e2[:, :, 0, :], start=(g == 0),
                             stop=(g == NCC - 1), perf_mode=DR)
            nc.tensor.matmul(att_b, v8b[:, 2 * g:2 * g + 2, 0:HD + 1],
                             e2[:, :, 1, :], start=(g == 0),
                             stop=(g == NCC - 1), perf_mode=DR)

        pend = None
        e2 = None
        for c in range(KC):
            if c % 2 == 0:
                e2 = exp_p.tile([128, 2, 2, TOK], F8, tag="e2")
            s = s_ps.tile([128, 2, TOK], F32, tag="s")
            nc.tensor.matmul(s[:, 0, :], kt_sb[0:64, ts(c, 128)],
                             qt_sb[0:64, :], tile_position=(0, 0))
            nc.tensor.matmul(s[:, 1, :], kt_sb[64:128, ts(c, 128)],
                             qt_sb[64:128, :], tile_position=(64, 0))
            if pend is not None:
                exp_c(*pend)
                if pend[0] & 1:
                    av_g(pend[0] // 2, pend[2])
            pend = (c, s, e2)
        exp_c(*pend)
        av_g(NCC - 1, e2)

        # ---- evacuate att PSUM to SBUF immediately (frees the att
        # banks so the next pair's AV accumulation can start), then
        # normalize by the ones-row denominators with a DVE divide ----
        asb = nrm_p.tile([HD + 1, TOK], F32, tag="asb")
        bsb = nrm_p.tile([HD + 1, TOK], F32, tag="bsb")
        nc.vector.tensor_copy(out=asb, in_=att_a)
        nc.vector.tensor_copy(out=bsb, in_=att_b)
        # denominators bounce through DRAM reshaped to [128, 8] so the
        # reciprocal uses all 128 DVE lanes (a [1, 512] reciprocal is
        # single-lane and costs ~4us)
        bcd = dram_p.tile([2, TOK], F32, tag="bcd")
        nc.sync.dma_start(out=bcd[0:1, :], in_=asb[HD:HD + 1, :])
        nc.sync.dma_start(out=bcd[1:2, :], in_=bsb[HD:HD + 1, :])
        rcp = nrm_p.tile([128, 2 * TOK // 128], F32, tag="rcp")
        nc.sync.dma_start(
            out=rcp, in_=bcd[:, :].rearrange("a (g c) -> (a g) c", c=2 * TOK // 128))
        nc.vector.reciprocal(rcp, rcp)
        bcd2 = dram_p.tile([2, TOK], F32, tag="bcd2")
        nc.sync.dma_start(
            out=bcd2[:, :].rearrange("a (g c) -> (a g) c", c=2 * TOK // 128),
            in_=rcp)
        bca = nrm_p.tile([64, TOK], F32, tag="bca")
        bcb = nrm_p.tile([64, TOK], F32, tag="bcb")
        nc.sync.dma_start(out=bca, in_=bcd2[0:1, :].to_broadcast((64, TOK)))
        nc.sync.dma_start(out=bcb, in_=bcd2[1:2, :].to_broadcast((64, TOK)))
        natt = nrm_p.tile([128, TOK], F32, tag="natt")
        tmpb = nrm_p.tile([64, TOK], F32, tag="tmpb")
        nc.vector.tensor_mul(natt[0:64, :], asb[0:HD, :], bca)
        nc.vector.tensor_mul(tmpb, bsb[0:HD, :], bcb)
        nc.sync.dma_start(out=natt[64:128, :], in_=tmpb)
        nc.vector.tensor_add(x1T[j], natt, xT[:, j, :])

        # flush the previous pair's layer-norm stats (deferred so the PE
        # queue at the pair boundary is not blocked behind the DVE square)
        if pend_sq is not None:
            emit_stats(j - 1, pend_sq)
        pend_sq = True
        if j + 1 < NPAIR:
            ready = ready_next

    sq_last = sq_p.tile([128, TOK], F32R, tag="sq")
    nc.vector.tensor_mul(sq_last, x1T[NPAIR - 1], x1T[NPAIR - 1])
    emit_stats(NPAIR - 1, sq_last)

    if DEBUG_X1:
        for j in range(DC):
            nc.sync.dma_start(out=x1dbg[j].bitcast(F32R), in_=x1T[j])

    # ---- layer-norm scalars ----
    mu = persist.tile([1, TOK], F32, tag="mu")
    msq = persist.tile([1, TOK], F32, tag="msq")
    var = persist.tile([1, TOK], F32, tag="var")
    rstd = persist.tile([1, TOK], F32, tag="rstd")
    nc.vector.tensor_scalar_mul(mu, stats[:, 0, :], 1.0 / D)
    nc.vector.tensor_scalar_mul(msq, stats[:, 1, :], 1.0 / D)
    st_est.close()
    nc.vector.tensor_mul(var, mu, mu)
    nc.vector.tensor_sub(var, msq, var)
    # rstd = exp(-0.5 * ln(var + eps)) -- stays within the ln/exp table set
    nc.scalar.activation(var, var, AF.Ln, bias=eps_t)
    nc.scalar.activation(rstd, var, AF.Exp, scale=-0.5)

    mu_b = persist.tile([128, TOK], F32, tag="mu_b")
    rstd_b = persist.tile([128, TOK], F32, tag="rstd_b")
    lnd = dram_p.tile([2, TOK], F32, tag="lnd")
    nc.sync.dma_start(out=lnd[0:1, :], in_=mu)
    nc.sync.dma_start(out=lnd[1:2, :], in_=rstd)
    nc.sync.dma_start(out=mu_b, in_=lnd[0:1, :].to_broadcast((128, TOK)))
    nc.sync.dma_start(out=rstd_b, in_=lnd[1:2, :].to_broadcast((128, TOK)))

    att_est.close()

    # =================== FFN ===================
    ffn_est = ExitStack()
    mm_ps = ffn_est.enter_context(tc.tile_pool(name="mm_ps", bufs=4, space="PSUM"))
    ln_p = ffn_est.enter_context(tc.tile_pool(name="ln", bufs=2))
    g_p = ffn_est.enter_context(tc.tile_pool(name="gp", bufs=1))
    h_p = ffn_est.enter_context(tc.tile_pool(name="hp", bufs=1))
    o_p = ffn_est.enter_context(tc.tile_pool(name="op", bufs=2))

    w1_p = ffn_est.enter_context(tc.tile_pool(name="w1p", bufs=4))

    # LN on DVE: h = ((x1 - mu) * lnw) * rstd + lnb, bf16
    hT = [h_p.tile([128, TOK], BF16, name=f"hT{j}", tag=f"hT{j}")
          for j in range(DC)]
    for j in range(DC):
        t = ln_p.tile([128, TOK], F32, tag="lnt")
        nc.vector.tensor_sub(t, x1T[j], mu_b)
        nc.vector.scalar_tensor_tensor(
            t, t, lnw_sb[:, j:j + 1], rstd_b, op0=ALU.mult, op1=ALU.mult)
        nc.vector.tensor_scalar_add(hT[j], t, lnb_sb[:, j:j + 1])

    gT = [g_p.tile([128, TOK], BF16, name=f"gT{f}", tag=f"gT{f}")
          for f in range(FC)]
    for f in range(FC):
        w1s = w1_p.tile([128, DC // 2, 128], F32, tag="w1s")
        nc.sync.dma_start(out=w1s[:, 0:2, :],
                          in_=w1_v[:, 0:2, ts(f, 128)])
        nc.scalar.dma_start(out=w1s[:, 2:4, :],
                            in_=w1_v[:, 2:4, ts(f, 128)])
        w1c = w1_p.tile([128, DC, 128], BF16, tag="w1c")
        nc.gpsimd.dma_start(out=w1c[:, DC // 2:, :],
                            in_=w1_v[:, DC // 2:, ts(f, 128)])
        nc.vector.tensor_copy(out=w1c[:, 0:DC // 2, :], in_=w1s)
        ps = mm_ps.tile([128, TOK], F32, tag="mm")
        for dc in range(DC):
            nc.tensor.matmul(ps, w1c[:, dc, :], hT[dc],
                             start=(dc == 0), stop=(dc == DC - 1))
        nc.scalar.activation(gT[f], ps, AF.Gelu, bias=b1_sb[:, f:f + 1])

    for dd in range(DC):
        ps = mm_ps.tile([128, TOK], F32, tag="mm")
        for fc in range(FC):
            nc.tensor.matmul(ps, w2b[:, fc, ts(dd, 128)], gT[fc],
                             start=(fc == 0), stop=(fc == FC - 1))
        yt = o_p.tile([128, TOK], F32, tag="yt")
        nc.vector.scalar_tensor_tensor(
            yt, ps, b2_sb[:, dd:dd + 1], x1T[dd], op0=ALU.add, op1=ALU.add)
        nc.sync.dma_start(out=y_v[:, dd, :], in_=yt)

    ffn_est.close()
    est.close()


_PROGRAMS = {}


def get_program(split_waits=True):
    if split_waits not in _PROGRAMS:
        _PROGRAMS[split_waits] = build_program(split_waits)
    return _PROGRAMS[split_waits]


def make_in_maps(x, image_q, image_k, image_v, ln_w, ln_b, w1, b1, w2, b2):
    asf = lambda a: np.ascontiguousarray(np.asarray(a, dtype=np.float32))
    x = np.asarray(x, dtype=np.float32)
    image_q = np.asarray(image_q, dtype=np.float32)
    image_k = np.asarray(image_k, dtype=np.float32)
    image_v = np.asarray(image_v, dtype=np.float32)
    shared = {
        "w1": asf(w1), "b1": asf(b1), "w2": asf(w2), "b2": asf(b2),
        "lnw": asf(ln_w), "lnb": asf(ln_b),
    }
    # per batch: feature-major K pairs [NPAIR, 128, N]
    ktb = [asf(image_k[b].transpose(0, 2, 1).reshape(NPAIR, 128, N))
           for b in range(B)]
    # V chunk layout matching contiguous score chunks: [H, p, c, d],
    # token = c*128 + p
    vb = [asf(image_v[b].reshape(H, KC, 128, HD).transpose(0, 2, 1, 3))
          for b in range(B)]
    in_maps = []
    for core in range(NCORES):
        b, r = divmod(core, NCORES // B)
        rows = slice(TOK * r, TOK * (r + 1))
        in_maps.append({
            "xt": asf(x[b, rows].T),
            "qt": asf(image_q[b, :, rows].transpose(0, 2, 1).reshape(
                NPAIR, 128, TOK)),
            "kt": ktb[b],
            "vs": vb[b],
            **shared,
        })
    return in_maps


def run_cores(in_maps, trace=False, **kw):
    nc = get_program()
    return run_bass_kernel_spmd(nc, in_maps, core_ids=list(range(NCORES)),
                                trace=trace, **kw)


def kernel(x, image_q, image_k, image_v, ln_w, ln_b, w1, b1, w2, b2):
    in_maps = make_in_maps(x, image_q, image_k, image_v, ln_w, ln_b,
                           w1, b1, w2, b2)
    res = run_cores(in_maps)
    out = np.empty((B, N, D), dtype=np.float32)
    for core in range(NCORES):
        b, r = divmod(core, NCORES // B)
        out[b, TOK * r:TOK * (r + 1)] = res.results[core]["y"].T
    return out
